# revision 33
# baseline (speedup 1.0000x reference)
"""Trainium2 Bass kernel for nn_CayleyNet (gnn_message_passing), 8 NeuronCores.

Strategy (graph/data parallel, per sharding hint):
- Nodes sharded 2500/core (padded to 2560 = 20 tiles x 128 partitions).
- Edges partitioned by scatter-destination; per destination-tile groups of
  edge slots (host-sorted/padded). Two orderings: O1 (scatter=row,
  gather=col; used by the B apply) and O2 (scatter=col, gather=row; Jacobi).
- CayleyNet edge weights depend only on one endpoint (tmp_left[row]), so every
  sparse op is an *unweighted* adjacency apply + per-node complex scalings:
      B y = -h*tl (.) (A1 @ y) + b_dia (.) y
      Jacobi: yk' = A2 @ (h*tl (.) yk) + b_j     (and g.(s.u+d.y) == g.b)
- The fp8e4 node-state table is SPLIT IN TWO HALVES (each core's local tiles
  0-9 -> table A, tiles 10-19 -> table B) with two AllGathers per
  propagation. Edge slots per dst tile are bucketed by source half
  (GTA+GTB 128-slot chunks). AG_A only needs the previous prop's first ten
  tile combines, so prop p's A-phase gathers overlap prop p-1's tail --
  the collective leaves the serial critical path.
- dma_gather on 4 SWDGE queues round-robin (each queue's descriptor
  generation runs on its own Q7 pair) -> one-hot S (fp8) matmuls on TensorE
  (segment-sum into PSUM, f32) -> fused DVE combines.
- Dense W / Wc matmuls in bf16; feature-major x provided by host (xt0);
  bf16 y staging (yb) feeds PE transposes for the Wc terms.
- Device computes x2 (feature-major, f32). Host does tanh-score / top-k /
  weighted mean / final linear (~0.25% of FLOPs; top-k selection).
"""
import numpy as np
import ml_dtypes

import concourse.bass as bass
import concourse.bacc as bacc
import concourse.mybir as mybir
import concourse.tile as tile
from concourse.bass_utils import run_bass_kernel_spmd

# ---- problem constants (hardcoded per spec) ----
N = 20000
E = 320000
FEAT = 128
HID = 128
OUT = 10
R = 2
K = 3
RATIO = 0.9
NCORES = 8
NLOC = 2500
NT = 20                  # node tiles per core
NTA = 10                 # tiles in table half A (B gets NT - NTA)
NLOC_PAD = NT * 128      # 2560
NLOC_A = NTA * 128       # 1280
ZROWS_H = NCORES * NLOC_A
F = 128                  # feature width
F2 = 2 * F               # re||im row width of the z table
ET = 128                 # edges per tile

BF16 = mybir.dt.bfloat16
FP8 = mybir.dt.float8e4
F32 = mybir.dt.float32
I16 = mybir.dt.int16
FP8NP = ml_dtypes.float8_e4m3

_cache = {}


# ----------------------------------------------------------------------------
# host preprocessing
# ----------------------------------------------------------------------------

def _zrow_ab(gid):
    """(region, half-table row) for global node id; region 0 = local tiles
    0-9 (table A), region 1 = tiles 10-19 (table B)."""
    c = gid // NLOC
    l = gid - c * NLOC
    reg = (l >= NLOC_A).astype(np.int64)
    return reg, c * NLOC_A + (l - reg * NLOC_A)


def _build_edge_tables(row, col):
    """Per ordering/core: A/B gather-idx (wrapped int16) + one-hot S with
    A-chunks then B-chunks per dst tile."""
    maxa = maxb = 0
    for dst, src in ((row, col), (col, row)):
        regs = (src % NLOC) >= NLOC_A
        loc = dst % NLOC
        gid = (dst // NLOC) * NT + loc // 128
        cnta = np.bincount(gid[~regs], minlength=NCORES * NT)
        cntb = np.bincount(gid[regs], minlength=NCORES * NT)
        maxa = max(maxa, int(cnta.max()))
        maxb = max(maxb, int(cntb.max()))
    GTA = int(np.ceil(maxa / ET))
    GTB = int(np.ceil(maxb / ET))
    GTS = GTA + GTB

    tabs = {}
    for name, dst, src in (("O1", row, col), ("O2", col, row)):
        order = np.argsort(dst, kind="stable")
        dst_s, src_s = dst[order], src[order]
        reg, zr = _zrow_ab(src_s)
        idxa_all = np.zeros((NCORES, NT * GTA * ET), np.int32)
        idxb_all = np.zeros((NCORES, NT * GTB * ET), np.int32)
        S_all = np.zeros((NCORES, NT * GTS, ET, ET), np.float32)
        d_loc = dst_s % NLOC
        d_core = dst_s // NLOC
        d_tile = d_loc // 128
        d_slot = d_loc - d_tile * 128
        for c in range(NCORES):
            m = d_core == c
            dt_c, ds_c, zr_c, rg_c = d_tile[m], d_slot[m], zr[m], reg[m]
            for g in range(NT):
                gm = dt_c == g
                for rgn, idx_all, GTX, coff in ((0, idxa_all, GTA, 0),
                                                (1, idxb_all, GTB, GTA)):
                    sel = gm & (rg_c == rgn)
                    nd = int(sel.sum())
                    assert nd <= GTX * ET
                    idx_all[c, g * GTX * ET: g * GTX * ET + nd] = zr_c[sel]
                    ks = np.arange(nd)
                    S_all[c, g * GTS + coff + ks // ET, ks % ET,
                          ds_c[sel]] = 1.0

        def wrap(a):
            w = a.reshape(NCORES, -1, 16).transpose(0, 2, 1)
            return np.tile(w, (1, 8, 1)).astype(np.int16)

        S_flat = (S_all.reshape(NCORES, NT, GTS, ET, ET)
                  .transpose(0, 1, 3, 2, 4)
                  .reshape(NCORES, NT, ET, GTS * ET)
                  .astype(FP8NP))
        tabs[name] = (wrap(idxa_all), wrap(idxb_all), S_flat)
    return GTA, GTB, tabs


def _shard_cols(v):
    """[N] -> [NCORES, 128, NT] per-node columns (node (t,p) -> [:, p, t])."""
    pad = np.zeros(NCORES * NLOC_PAD, np.float32)
    for c in range(NCORES):
        pad[c * NLOC_PAD: c * NLOC_PAD + NLOC] = v[c * NLOC: (c + 1) * NLOC]
    lp = pad.reshape(NCORES, NT, 128)
    return lp.transpose(0, 2, 1).copy()


def _node_major(x):
    """[N, F] f32 -> [NCORES, 128, NT, F]: node (t,p) at [c, p, t, :]."""
    out = np.zeros((NCORES, NT, 128, x.shape[1]), np.float32)
    for c in range(NCORES):
        out[c].reshape(NLOC_PAD, -1)[:NLOC] = x[c * NLOC:(c + 1) * NLOC]
    return out.transpose(0, 2, 1, 3).copy()


def _scales(deg, h, alpha):
    """Per-conv per-node scale columns: s = B post-scale, d = b_dia,
    g = Jacobi pre-scale."""
    l = (deg - alpha).astype(np.float64)
    tl = 1.0 / (h * l + 1j)
    s = -h * tl
    d = tl * (h * l - 1j)
    g = h * tl
    out = {}
    for nm, v in (("s", s), ("d", d), ("g", g)):
        out[nm + "_re"] = np.real(v)
        out[nm + "_im"] = np.imag(v)
        out["n" + nm + "_im"] = -np.imag(v)
    return out


SCAL_NAMES = ["s_re", "s_im", "ns_im", "d_re", "d_im", "nd_im",
              "g_re", "g_im", "ng_im"]
NSCAL = len(SCAL_NAMES)


# ----------------------------------------------------------------------------
# kernel builder
# ----------------------------------------------------------------------------

def _build(GTA, GTB):
    GTS = GTA + GTB
    ICA = NT * GTA * ET // 16     # idx cols per ordering, table A
    ICB = NT * GTB * ET // 16
    GCA = GTA * ET // 16          # idx cols per dst-tile group
    GCB = GTB * ET // 16

    nc = bacc.Bacc("TRN2", target_bir_lowering=False, debug=False,
                   num_devices=NCORES, num_swdge_queues=4)

    xz_in = nc.dram_tensor("xz", [128, NT, F2], FP8, kind="ExternalInput")
    xt0_in = nc.dram_tensor("xt0", [128, NT * F], BF16, kind="ExternalInput")
    zt0a_in = nc.dram_tensor("zt0a", [ZROWS_H, F2], FP8, kind="ExternalInput")
    zt0b_in = nc.dram_tensor("zt0b", [ZROWS_H, F2], FP8, kind="ExternalInput")
    y0_in = nc.dram_tensor("y0", [128, NT * F], F32, kind="ExternalInput")
    i1a_in = nc.dram_tensor("i1a", [128, ICA], I16, kind="ExternalInput")
    i1b_in = nc.dram_tensor("i1b", [128, ICB], I16, kind="ExternalInput")
    i2a_in = nc.dram_tensor("i2a", [128, ICA], I16, kind="ExternalInput")
    i2b_in = nc.dram_tensor("i2b", [128, ICB], I16, kind="ExternalInput")
    s1_in = nc.dram_tensor("s1", [NT, 128, GTS * ET], FP8, kind="ExternalInput")
    s2_in = nc.dram_tensor("s2", [NT, 128, GTS * ET], FP8, kind="ExternalInput")
    scal_in = nc.dram_tensor("scal", [128, 2 * NSCAL * NT], F32, kind="ExternalInput")
    wts_in = nc.dram_tensor("wts", [128, 10 * 128], BF16, kind="ExternalInput")
    ident_in = nc.dram_tensor("ident", [128, 128], BF16, kind="ExternalInput")
    xfeat_out = nc.dram_tensor("xfeat", [128, NT * F], F32, kind="ExternalOutput")

    s_dram = {"O1": s1_in, "O2": s2_in}

    with tile.TileContext(nc) as tc:
        with (
            tc.tile_pool(name="persist", bufs=1) as pp,
            tc.tile_pool(name="gpa", bufs=NT) as gpa,
            tc.tile_pool(name="gpb", bufs=8) as gpb,
            tc.tile_pool(name="spool", bufs=6) as spool,
            tc.tile_pool(name="tmp", bufs=4) as tmpp,
            tc.tile_pool(name="prop_ps", bufs=5, space="PSUM") as prop_ps,
            tc.tile_pool(name="tr_ps", bufs=2, space="PSUM") as tr_ps,
            tc.tile_pool(name="mm_ps", bufs=1, space="PSUM") as mm_ps,
            tc.tile_pool(name="dram", bufs=1, space="DRAM") as dram,
        ):
            # ---- persistent SBUF state ----
            z_own = pp.tile([128, NT, F2], FP8)         # fp8 node state (re||im)
            yb = pp.tile([128, NT, F2], BF16)           # bf16 y staging (re||im)
            y_re = pp.tile([128, NT, F], F32)
            y_im = pp.tile([128, NT, F], F32)
            b_re = pp.tile([128, NT, F], F32)
            b_im = pp.tile([128, NT, F], F32)
            out_acc = pp.tile([128, NT * F], F32)       # feature-major conv accum
            # r_bf doubles as feature-major bf16 conv input: xt0 for conv1,
            # then relu(x1) for conv2
            r_bf = pp.tile([128, NT * F], BF16, name="rbf")
            yT_re = pp.tile([128, NT * F], BF16)
            yT_im = pp.tile([128, NT * F], BF16)
            idx_a = {"O1": pp.tile([128, ICA], I16, name="i1a_sb"),
                     "O2": pp.tile([128, ICA], I16, name="i2a_sb")}
            idx_b = {"O1": pp.tile([128, ICB], I16, name="i1b_sb"),
                     "O2": pp.tile([128, ICB], I16, name="i2b_sb")}
            scal_sb = pp.tile([128, 2 * NSCAL * NT], F32)
            wts_sb = pp.tile([128, 10 * 128], BF16)
            ident = pp.tile([128, 128], BF16)

            zin_a = dram.tile([NLOC_A, F2], FP8)
            zin_b = dram.tile([NLOC_A, F2], FP8)
            NPROPS = 2 * R * (1 + K)
            ztabA = [zt0a_in] + [
                dram.tile([ZROWS_H, F2], FP8, addr_space="Shared",
                          name=f"ztA{i}") for i in range(1, NPROPS)]
            ztabB = [zt0b_in] + [
                dram.tile([ZROWS_H, F2], FP8, addr_space="Shared",
                          name=f"ztB{i}") for i in range(1, NPROPS)]

            # ---- load constants ----
            nc.sync.dma_start(idx_a["O1"][:], i1a_in[:])
            nc.sync.dma_start(idx_b["O1"][:], i1b_in[:])
            nc.sync.dma_start(idx_a["O2"][:], i2a_in[:])
            nc.sync.dma_start(idx_b["O2"][:], i2b_in[:])
            nc.sync.dma_start(scal_sb[:], scal_in[:])
            nc.sync.dma_start(wts_sb[:], wts_in[:])
            nc.sync.dma_start(ident[:], ident_in[:])
            nc.sync.dma_start(z_own[:], xz_in[:])
            nc.sync.dma_start(r_bf[:], xt0_in[:])
            nc.sync.dma_start(y_re[:], y0_in[:])
            nc.vector.memset(y_im[:], 0.0)

            def col(ci, name, t):
                k = ci * NSCAL + SCAL_NAMES.index(name)
                return scal_sb[:, k * NT + t: k * NT + t + 1]

            def wt(k):
                return wts_sb[:, k * 128:(k + 1) * 128]

            def prop(pi, ordering, consumer):
                """AG_A -> A gathers (all 20) -> AG_B -> per tile: B gather +
                S matmuls (A chunks then B chunks) -> consumer."""
                if pi > 0:
                    nc.gpsimd.collective_compute(
                        "AllGather", mybir.AluOpType.bypass,
                        replica_groups=[list(range(NCORES))],
                        ins=[zin_a.opt()], outs=[ztabA[pi].opt()],
                    )
                gbsA = []
                for g in range(NT):
                    gba = gpa.tile([128, GTA, F2], FP8, tag="gba")
                    nc.gpsimd.dma_gather(
                        gba[:], ztabA[pi][:],
                        idx_a[ordering][:, g * GCA:(g + 1) * GCA],
                        num_idxs=GTA * ET, num_idxs_reg=GTA * ET,
                        elem_size=F2, single_packet=False, queue_num=g % 4,
                    )
                    gbsA.append(gba)
                if pi > 0:
                    nc.gpsimd.collective_compute(
                        "AllGather", mybir.AluOpType.bypass,
                        replica_groups=[list(range(NCORES))],
                        ins=[zin_b.opt()], outs=[ztabB[pi].opt()],
                    )
                for g in range(NT):
                    gbb = gpb.tile([128, GTB, F2], FP8, tag="gbb")
                    nc.gpsimd.dma_gather(
                        gbb[:], ztabB[pi][:],
                        idx_b[ordering][:, g * GCB:(g + 1) * GCB],
                        num_idxs=GTB * ET, num_idxs_reg=GTB * ET,
                        elem_size=F2, single_packet=False, queue_num=g % 4,
                    )
                    ssb = spool.tile([128, GTS * ET], FP8, tag="schunk")
                    nc.sync.dma_start(ssb[:], s_dram[ordering][g])
                    ps = prop_ps.tile([128, F2], F32, tag="prop_ps")
                    for t in range(GTA):
                        nc.tensor.matmul(ps[:], ssb[:, t * ET:(t + 1) * ET],
                                         gbsA[g][:, t, :],
                                         start=(t == 0), stop=False)
                    for t in range(GTB):
                        nc.tensor.matmul(
                            ps[:], ssb[:, (GTA + t) * ET:(GTA + t + 1) * ET],
                            gbb[:, t, :],
                            start=False, stop=(t == GTB - 1))
                    consumer(g, ps)

            STT = nc.vector.scalar_tensor_tensor
            MUL = mybir.AluOpType.mult
            ADD = mybir.AluOpType.add
            COPY = mybir.ActivationFunctionType.Copy

            def smul(out_ap, in_ap, c_ap):
                """out = in * per-partition scalar, on the (idle) Scalar engine."""
                nc.scalar.activation(out_ap, in_ap, COPY, scale=c_ap)

            def zin_tile(g):
                """Push this dst tile's fresh z rows to its half-table input."""
                if g < NTA:
                    nc.sync.dma_start(zin_a[g * 128:(g + 1) * 128, :],
                                      z_own[:, g, :])
                else:
                    gg = g - NTA
                    nc.sync.dma_start(zin_b[gg * 128:(gg + 1) * 128, :],
                                      z_own[:, g, :])

            def b_consumer(ci):
                def consume(g, ps):
                    u_re, u_im = ps[:, 0:F], ps[:, F:F2]
                    # b = s.u + d.y
                    tmp3 = tmpp.tile([128, F], F32, tag="ctmp3")
                    smul(tmp3[:], u_re, col(ci, "s_re", g))
                    STT(tmp3[:], u_im, col(ci, "ns_im", g), tmp3[:], MUL, ADD)
                    STT(tmp3[:], y_re[:, g, :], col(ci, "d_re", g), tmp3[:], MUL, ADD)
                    STT(b_re[:, g, :], y_im[:, g, :], col(ci, "nd_im", g), tmp3[:], MUL, ADD)
                    tmp4 = tmpp.tile([128, F], F32, tag="ctmp4")
                    smul(tmp4[:], u_im, col(ci, "s_re", g))
                    STT(tmp4[:], u_re, col(ci, "s_im", g), tmp4[:], MUL, ADD)
                    STT(tmp4[:], y_im[:, g, :], col(ci, "d_re", g), tmp4[:], MUL, ADD)
                    STT(b_im[:, g, :], y_re[:, g, :], col(ci, "d_im", g), tmp4[:], MUL, ADD)
                    # first Jacobi gather operand: gs.u + gd.y == g.(s.u+d.y) = g.b
                    tmp = tmpp.tile([128, F], F32, tag="ctmp")
                    smul(tmp[:], b_re[:, g, :], col(ci, "g_re", g))
                    STT(z_own[:, g, 0:F], b_im[:, g, :], col(ci, "ng_im", g), tmp[:], MUL, ADD)
                    tmp2 = tmpp.tile([128, F], F32, tag="ctmp2")
                    smul(tmp2[:], b_im[:, g, :], col(ci, "g_re", g))
                    STT(z_own[:, g, F:F2], b_re[:, g, :], col(ci, "g_im", g), tmp2[:], MUL, ADD)
                    zin_tile(g)
                return consume

            def jacobi_consumer(ci, last, need_z):
                def consume(g, ps):
                    u_re, u_im = ps[:, 0:F], ps[:, F:F2]
                    # y = u + b   (this is yk)
                    nc.vector.tensor_tensor(y_re[:, g, :], u_re, b_re[:, g, :], ADD)
                    nc.vector.tensor_tensor(y_im[:, g, :], u_im, b_im[:, g, :], ADD)
                    if last:
                        if need_z:
                            # next B apply gathers z = y
                            smul(z_own[:, g, 0:F], y_re[:, g, :], 1.0)
                            smul(z_own[:, g, F:F2], y_im[:, g, :], 1.0)
                            zin_tile(g)
                        # bf16 staging for the Wc transposes
                        smul(yb[:, g, 0:F], y_re[:, g, :], 1.0)
                        smul(yb[:, g, F:F2], y_im[:, g, :], 1.0)
                    else:
                        # z = g (.) y   (next Jacobi gather operand)
                        tmp = tmpp.tile([128, F], F32, tag="ctmp")
                        smul(tmp[:], y_re[:, g, :], col(ci, "g_re", g))
                        STT(z_own[:, g, 0:F], y_im[:, g, :], col(ci, "ng_im", g), tmp[:], MUL, ADD)
                        tmp2 = tmpp.tile([128, F], F32, tag="ctmp2")
                        smul(tmp2[:], y_im[:, g, :], col(ci, "g_re", g))
                        STT(z_own[:, g, F:F2], y_re[:, g, :], col(ci, "g_im", g), tmp2[:], MUL, ADD)
                        zin_tile(g)
                return consume

            def transpose_to(dst, src_ap, t):
                """dst[:, t*128:(t+1)*128] = src_ap.T (both bf16)."""
                pt = tr_ps.tile([128, 128], BF16, tag="trps")
                nc.tensor.transpose(pt[:], src_ap, ident[:])
                nc.vector.tensor_copy(dst[:, t * 128:(t + 1) * 128], pt[:])

            def dense_chunks(lhs_ks, rhs_list, first):
                """out_acc[:, ch] (+)= sum_i lhsT(k_i) @ rhs_i[:, ch] (x2 if not first)."""
                nch = NT * F // 512
                for ch in range(nch):
                    sl = slice(ch * 512, (ch + 1) * 512)
                    ps = mm_ps.tile([128, 512], F32, tag="mmps")
                    for i, (k, rhs) in enumerate(zip(lhs_ks, rhs_list)):
                        nc.tensor.matmul(ps[:], wt(k), rhs[:, sl],
                                         start=(i == 0), stop=(i == len(lhs_ks) - 1))
                    if first:
                        nc.vector.tensor_copy(out_acc[:, sl], ps[:])
                    else:
                        STT(out_acc[:, sl], ps[:], 2.0, out_acc[:, sl], MUL, ADD)

            # ================= conv block =================
            pctr = [0]
            for ci in range(2):
                wbase = ci * 5
                dense_chunks([wbase + 0], [r_bf], first=True)

                for j in range(R):
                    prop(pctr[0], "O1", b_consumer(ci)); pctr[0] += 1
                    for it in range(K):
                        prop(pctr[0], "O2",
                             jacobi_consumer(ci, last=(it == K - 1),
                                             need_z=(j < R - 1)))
                        pctr[0] += 1
                    # yT from yb halves (bf16 copies of y)
                    for t in range(NT):
                        transpose_to(yT_re, yb[:, t, 0:F], t)
                        transpose_to(yT_im, yb[:, t, F:F2], t)
                    dense_chunks([wbase + 1 + 2 * j, wbase + 2 + 2 * j],
                                 [yT_re, yT_im], first=False)

                if ci == 0:
                    # relu -> bf16, transpose back to node-major, reseed state
                    nc.vector.tensor_scalar_max(r_bf[:], out_acc[:], 0.0)
                    nc.vector.memset(y_im[:], 0.0)
                    for t in range(NT):
                        pt = tr_ps.tile([128, 128], BF16, tag="trps")
                        nc.tensor.transpose(pt[:], r_bf[:, t * 128:(t + 1) * 128],
                                            ident[:])
                        nc.vector.tensor_copy(z_own[:, t, 0:F], pt[:])
                        nc.vector.memset(z_own[:, t, F:F2], 0.0)
                        nc.vector.tensor_copy(y_re[:, t, :], pt[:])
                        zin_tile(t)
                else:
                    # x2 = relu(out_acc) in place, feature-major f32 -> DRAM
                    nc.vector.tensor_scalar_max(out_acc[:], out_acc[:], 0.0)
                    nc.sync.dma_start(xfeat_out[:], out_acc[:])

    nc.compile()
    return nc


# ----------------------------------------------------------------------------
# entry point
# ----------------------------------------------------------------------------

def kernel(x, edge_index, W_real1, Wc1, W_real2, Wc2, h, alpha,
           pool_w, lin_W, lin_b):
    x = np.asarray(x, np.float32)
    edge_index = np.asarray(edge_index)
    row, col = edge_index[0].astype(np.int64), edge_index[1].astype(np.int64)

    GTA, GTB, tabs = _build_edge_tables(row, col)
    if "nc" not in _cache or _cache.get("GT") != (GTA, GTB):
        _cache["nc"] = _build(GTA, GTB)
        _cache["GT"] = (GTA, GTB)
    nc = _cache["nc"]

    deg = np.bincount(row, minlength=N).astype(np.float64)

    # per-node scale columns, both convs
    scal = np.zeros((NCORES, 128, 2 * NSCAL * NT), np.float32)
    for ci in range(2):
        sc = _scales(deg, float(np.asarray(h)[ci]), float(np.asarray(alpha)[ci]))
        for k, name in enumerate(SCAL_NAMES):
            cols = _shard_cols(sc[name].astype(np.float32))
            scal[:, :, (ci * NSCAL + k) * NT:(ci * NSCAL + k + 1) * NT] = cols

    # weights: lhsT layouts [cin, cout] bf16; imag pre-negated
    def T16(w):
        return np.ascontiguousarray(w.T).astype(ml_dtypes.bfloat16)
    wts = np.zeros((128, 10 * 128), ml_dtypes.bfloat16)
    packs = [T16(W_real1), T16(Wc1[0, :, :, 0]), T16(-Wc1[0, :, :, 1]),
             T16(Wc1[1, :, :, 0]), T16(-Wc1[1, :, :, 1]),
             T16(W_real2), T16(Wc2[0, :, :, 0]), T16(-Wc2[0, :, :, 1]),
             T16(Wc2[1, :, :, 0]), T16(-Wc2[1, :, :, 1])]
    for k, w in enumerate(packs):
        wts[:, k * 128:(k + 1) * 128] = w

    xn = _node_major(x)                                   # [NCORES,128,NT,F]
    xz = np.zeros((NCORES, 128, NT, F2), FP8NP)
    xz[:, :, :, :F] = xn.astype(FP8NP)
    y0 = xn.reshape(NCORES, 128, NT * F)
    # feature-major bf16 x per core: xt0[c][f, t*128+p] = x[node(t,p), f]
    xt0 = np.ascontiguousarray(
        xn.transpose(0, 3, 2, 1)                          # [c, F, NT, 128]
    ).reshape(NCORES, F, NT * 128).astype(ml_dtypes.bfloat16)
    # replicated full fp8 half-tables for prop 0 (skips the first AllGather):
    # half-row c*1280 + t*128 + p = node (c, t(+NTA), p), re||im
    full = xn.transpose(0, 2, 1, 3).reshape(NCORES, NT, 128, F)  # [c,t,p,F]
    zt0a = np.zeros((ZROWS_H, F2), FP8NP)
    zt0b = np.zeros((ZROWS_H, F2), FP8NP)
    zt0a[:, :F] = full[:, :NTA].reshape(ZROWS_H, F).astype(FP8NP)
    zt0b[:, :F] = full[:, NTA:].reshape(ZROWS_H, F).astype(FP8NP)

    ident = np.eye(128, dtype=ml_dtypes.bfloat16)

    (i1a, i1b, S1), (i2a, i2b, S2) = tabs["O1"], tabs["O2"]
    in_maps = []
    for c in range(NCORES):
        in_maps.append({
            "xz": xz[c], "xt0": xt0[c], "zt0a": zt0a, "zt0b": zt0b,
            "y0": y0[c],
            "i1a": i1a[c], "i1b": i1b[c], "i2a": i2a[c], "i2b": i2b[c],
            "s1": S1[c], "s2": S2[c],
            "scal": scal[c], "wts": wts, "ident": ident,
        })

    import os
    trace = os.environ.get("KERNEL_TRACE", "0") == "1"
    res = run_bass_kernel_spmd(nc, in_maps, core_ids=list(range(NCORES)),
                               trace=trace)
    _cache["last_results"] = res

    # unshard x2: xfeat[c][o, t*128+p] -> x2[c*2500 + t*128 + p, o]
    x2 = np.empty((N, HID), np.float32)
    for c in range(NCORES):
        xf = res.results[c]["xfeat"].reshape(128, NT * F)
        x2[c * NLOC:(c + 1) * NLOC] = xf.T[:NLOC]

    # host tail: tanh score, top-k (stable ties), weighted mean, linear
    pw = np.asarray(pool_w, np.float32)
    score = np.tanh((x2 @ pw) / np.linalg.norm(pw)).astype(np.float32)
    kpool = int(np.ceil(RATIO * N))
    idx = np.argsort(-score, kind="stable")[:kpool]
    x_sel = x2[idx] * score[idx][:, None]
    pooled = x_sel.mean(axis=0, keepdims=True).astype(np.float32)
    return (pooled @ np.asarray(lin_W, np.float32).T
            + np.asarray(lin_b, np.float32)).astype(np.float32)


# revision 34
# speedup vs baseline: 1.0198x; 1.0198x over previous
"""Trainium2 Bass kernel for nn_CayleyNet (gnn_message_passing), 8 NeuronCores.

Strategy (graph/data parallel, per sharding hint):
- Nodes sharded 2500/core (padded to 2560 = 20 tiles x 128 partitions).
- Edges partitioned by scatter-destination; per destination-tile groups of
  edge slots (host-sorted/padded). Two orderings: O1 (scatter=row,
  gather=col; used by the B apply) and O2 (scatter=col, gather=row; Jacobi).
- CayleyNet edge weights depend only on one endpoint (tmp_left[row]), so every
  sparse op is an *unweighted* adjacency apply + per-node complex scalings:
      B y = -h*tl (.) (A1 @ y) + b_dia (.) y
      Jacobi: yk' = A2 @ (h*tl (.) yk) + b_j     (and g.(s.u+d.y) == g.b)
- The fp8e4 node-state table is SPLIT IN TWO HALVES (each core's local tiles
  0-9 -> table A, tiles 10-19 -> table B) with two AllGathers per
  propagation. Edge slots per dst tile are bucketed by source half
  (GTA+GTB 128-slot chunks). AG_A only needs the previous prop's first ten
  tile combines, so prop p's A-phase gathers overlap prop p-1's tail --
  the collective leaves the serial critical path.
- dma_gather on 4 SWDGE queues round-robin (each queue's descriptor
  generation runs on its own Q7 pair) -> one-hot S (fp8) matmuls on TensorE
  (segment-sum into PSUM, f32) -> fused DVE combines.
- Dense W / Wc matmuls in bf16; feature-major x provided by host (xt0);
  bf16 y staging (yb) feeds PE transposes for the Wc terms.
- Device computes x2 (feature-major, f32). Host does tanh-score / top-k /
  weighted mean / final linear (~0.25% of FLOPs; top-k selection).
"""
import numpy as np
import ml_dtypes

import concourse.bass as bass
import concourse.bacc as bacc
import concourse.mybir as mybir
import concourse.tile as tile
from concourse.bass_utils import run_bass_kernel_spmd

# ---- problem constants (hardcoded per spec) ----
N = 20000
E = 320000
FEAT = 128
HID = 128
OUT = 10
R = 2
K = 3
RATIO = 0.9
NCORES = 8
NLOC = 2500
NT = 20                  # node tiles per core
NTA = 10                 # tiles in table half A (B gets NT - NTA)
NLOC_PAD = NT * 128      # 2560
NLOC_A = NTA * 128       # 1280
ZROWS_H = NCORES * NLOC_A
F = 128                  # feature width
F2 = 2 * F               # re||im row width of the z table
ET = 128                 # edges per tile

BF16 = mybir.dt.bfloat16
FP8 = mybir.dt.float8e4
F32 = mybir.dt.float32
I16 = mybir.dt.int16
FP8NP = ml_dtypes.float8_e4m3

_cache = {}


# ----------------------------------------------------------------------------
# host preprocessing
# ----------------------------------------------------------------------------

def _zrow_ab(gid):
    """(region, half-table row) for global node id; region 0 = local tiles
    0-9 (table A), region 1 = tiles 10-19 (table B)."""
    c = gid // NLOC
    l = gid - c * NLOC
    reg = (l >= NLOC_A).astype(np.int64)
    return reg, c * NLOC_A + (l - reg * NLOC_A)


def _build_edge_tables(row, col):
    """Per ordering/core: A/B gather-idx (wrapped int16) + one-hot S with
    A-chunks then B-chunks per dst tile."""
    maxa = maxb = 0
    for dst, src in ((row, col), (col, row)):
        regs = (src % NLOC) >= NLOC_A
        loc = dst % NLOC
        gid = (dst // NLOC) * NT + loc // 128
        cnta = np.bincount(gid[~regs], minlength=NCORES * NT)
        cntb = np.bincount(gid[regs], minlength=NCORES * NT)
        maxa = max(maxa, int(cnta.max()))
        maxb = max(maxb, int(cntb.max()))
    GTA = int(np.ceil(maxa / ET))
    GTB = int(np.ceil(maxb / ET))
    GTS = GTA + GTB

    tabs = {}
    for name, dst, src in (("O1", row, col), ("O2", col, row)):
        order = np.argsort(dst, kind="stable")
        dst_s, src_s = dst[order], src[order]
        reg, zr = _zrow_ab(src_s)
        idxa_all = np.zeros((NCORES, NT * GTA * ET), np.int32)
        idxb_all = np.zeros((NCORES, NT * GTB * ET), np.int32)
        S_all = np.zeros((NCORES, NT * GTS, ET, ET), np.float32)
        d_loc = dst_s % NLOC
        d_core = dst_s // NLOC
        d_tile = d_loc // 128
        d_slot = d_loc - d_tile * 128
        for c in range(NCORES):
            m = d_core == c
            dt_c, ds_c, zr_c, rg_c = d_tile[m], d_slot[m], zr[m], reg[m]
            for g in range(NT):
                gm = dt_c == g
                for rgn, idx_all, GTX, coff in ((0, idxa_all, GTA, 0),
                                                (1, idxb_all, GTB, GTA)):
                    sel = gm & (rg_c == rgn)
                    nd = int(sel.sum())
                    assert nd <= GTX * ET
                    idx_all[c, g * GTX * ET: g * GTX * ET + nd] = zr_c[sel]
                    ks = np.arange(nd)
                    S_all[c, g * GTS + coff + ks // ET, ks % ET,
                          ds_c[sel]] = 1.0

        def wrap(a):
            w = a.reshape(NCORES, -1, 16).transpose(0, 2, 1)
            return np.tile(w, (1, 8, 1)).astype(np.int16)

        S_flat = (S_all.reshape(NCORES, NT, GTS, ET, ET)
                  .transpose(0, 1, 3, 2, 4)
                  .reshape(NCORES, NT, ET, GTS * ET)
                  .astype(FP8NP))
        tabs[name] = (wrap(idxa_all), wrap(idxb_all), S_flat)
    return GTA, GTB, tabs


def _shard_cols(v):
    """[N] -> [NCORES, 128, NT] per-node columns (node (t,p) -> [:, p, t])."""
    pad = np.zeros(NCORES * NLOC_PAD, np.float32)
    for c in range(NCORES):
        pad[c * NLOC_PAD: c * NLOC_PAD + NLOC] = v[c * NLOC: (c + 1) * NLOC]
    lp = pad.reshape(NCORES, NT, 128)
    return lp.transpose(0, 2, 1).copy()


def _node_major(x):
    """[N, F] f32 -> [NCORES, 128, NT, F]: node (t,p) at [c, p, t, :]."""
    out = np.zeros((NCORES, NT, 128, x.shape[1]), np.float32)
    for c in range(NCORES):
        out[c].reshape(NLOC_PAD, -1)[:NLOC] = x[c * NLOC:(c + 1) * NLOC]
    return out.transpose(0, 2, 1, 3).copy()


def _scales(deg, h, alpha):
    """Per-conv per-node scale columns: s = B post-scale, d = b_dia,
    g = Jacobi pre-scale."""
    l = (deg - alpha).astype(np.float64)
    tl = 1.0 / (h * l + 1j)
    s = -h * tl
    d = tl * (h * l - 1j)
    g = h * tl
    out = {}
    for nm, v in (("s", s), ("d", d), ("g", g)):
        out[nm + "_re"] = np.real(v)
        out[nm + "_im"] = np.imag(v)
        out["n" + nm + "_im"] = -np.imag(v)
    return out


SCAL_NAMES = ["s_re", "s_im", "ns_im", "d_re", "d_im", "nd_im",
              "g_re", "g_im", "ng_im"]
NSCAL = len(SCAL_NAMES)


# ----------------------------------------------------------------------------
# kernel builder
# ----------------------------------------------------------------------------

def _build(GTA, GTB):
    GTS = GTA + GTB
    ICA = NT * GTA * ET // 16     # idx cols per ordering, table A
    ICB = NT * GTB * ET // 16
    GCA = GTA * ET // 16          # idx cols per dst-tile group
    GCB = GTB * ET // 16

    nc = bacc.Bacc("TRN2", target_bir_lowering=False, debug=False,
                   num_devices=NCORES, num_swdge_queues=4)

    xz_in = nc.dram_tensor("xz", [128, NT, F2], FP8, kind="ExternalInput")
    xt0_in = nc.dram_tensor("xt0", [128, NT * F], BF16, kind="ExternalInput")
    zt0a_in = nc.dram_tensor("zt0a", [ZROWS_H, F2], FP8, kind="ExternalInput")
    zt0b_in = nc.dram_tensor("zt0b", [ZROWS_H, F2], FP8, kind="ExternalInput")
    y0_in = nc.dram_tensor("y0", [128, NT * F], F32, kind="ExternalInput")
    i1a_in = nc.dram_tensor("i1a", [128, ICA], I16, kind="ExternalInput")
    i1b_in = nc.dram_tensor("i1b", [128, ICB], I16, kind="ExternalInput")
    i2a_in = nc.dram_tensor("i2a", [128, ICA], I16, kind="ExternalInput")
    i2b_in = nc.dram_tensor("i2b", [128, ICB], I16, kind="ExternalInput")
    s1_in = nc.dram_tensor("s1", [NT, 128, GTS * ET], FP8, kind="ExternalInput")
    s2_in = nc.dram_tensor("s2", [NT, 128, GTS * ET], FP8, kind="ExternalInput")
    scal_in = nc.dram_tensor("scal", [128, 2 * NSCAL * NT], F32, kind="ExternalInput")
    wts_in = nc.dram_tensor("wts", [128, 10 * 128], BF16, kind="ExternalInput")
    ident_in = nc.dram_tensor("ident", [128, 128], BF16, kind="ExternalInput")
    xfeat_out = nc.dram_tensor("xfeat", [128, NT * F], F32, kind="ExternalOutput")

    s_dram = {"O1": s1_in, "O2": s2_in}

    with tile.TileContext(nc) as tc:
        with (
            tc.tile_pool(name="persist", bufs=1) as pp,
            tc.tile_pool(name="gpa", bufs=NT) as gpa,
            tc.tile_pool(name="gpb", bufs=10) as gpb,
            tc.tile_pool(name="spool", bufs=6) as spool,
            tc.tile_pool(name="tmp", bufs=4) as tmpp,
            tc.tile_pool(name="prop_ps", bufs=5, space="PSUM") as prop_ps,
            tc.tile_pool(name="tr_ps", bufs=2, space="PSUM") as tr_ps,
            tc.tile_pool(name="mm_ps", bufs=1, space="PSUM") as mm_ps,
            tc.tile_pool(name="dram", bufs=1, space="DRAM") as dram,
        ):
            # ---- persistent SBUF state ----
            z_own = pp.tile([128, NT, F2], FP8)         # fp8 node state (re||im)
            yb = pp.tile([128, NT, F2], BF16)           # bf16 y staging (re||im)
            y_re = pp.tile([128, NT, F], F32)
            y_im = pp.tile([128, NT, F], F32)
            b_re = pp.tile([128, NT, F], F32)
            b_im = pp.tile([128, NT, F], F32)
            out_acc = pp.tile([128, NT * F], F32)       # feature-major conv accum
            # r_bf doubles as feature-major bf16 conv input: xt0 for conv1,
            # then relu(x1) for conv2
            r_bf = pp.tile([128, NT * F], BF16, name="rbf")
            yT_re = pp.tile([128, NT * F], BF16)
            yT_im = pp.tile([128, NT * F], BF16)
            idx_a = {"O1": pp.tile([128, ICA], I16, name="i1a_sb"),
                     "O2": pp.tile([128, ICA], I16, name="i2a_sb")}
            idx_b = {"O1": pp.tile([128, ICB], I16, name="i1b_sb"),
                     "O2": pp.tile([128, ICB], I16, name="i2b_sb")}
            scal_sb = pp.tile([128, 2 * NSCAL * NT], F32)
            wts_sb = pp.tile([128, 10 * 128], BF16)
            ident = pp.tile([128, 128], BF16)

            zin_a = dram.tile([NLOC_A, F2], FP8)
            zin_b = dram.tile([NLOC_A, F2], FP8)
            NPROPS = 2 * R * (1 + K)
            ztabA = [zt0a_in] + [
                dram.tile([ZROWS_H, F2], FP8, addr_space="Shared",
                          name=f"ztA{i}") for i in range(1, NPROPS)]
            ztabB = [zt0b_in] + [
                dram.tile([ZROWS_H, F2], FP8, addr_space="Shared",
                          name=f"ztB{i}") for i in range(1, NPROPS)]

            # ---- load constants ----
            nc.sync.dma_start(idx_a["O1"][:], i1a_in[:])
            nc.sync.dma_start(idx_b["O1"][:], i1b_in[:])
            nc.sync.dma_start(idx_a["O2"][:], i2a_in[:])
            nc.sync.dma_start(idx_b["O2"][:], i2b_in[:])
            nc.sync.dma_start(scal_sb[:], scal_in[:])
            nc.sync.dma_start(wts_sb[:], wts_in[:])
            nc.sync.dma_start(ident[:], ident_in[:])
            nc.sync.dma_start(z_own[:], xz_in[:])
            nc.sync.dma_start(r_bf[:], xt0_in[:])
            nc.sync.dma_start(y_re[:], y0_in[:])
            nc.vector.memset(y_im[:], 0.0)

            def col(ci, name, t):
                k = ci * NSCAL + SCAL_NAMES.index(name)
                return scal_sb[:, k * NT + t: k * NT + t + 1]

            def wt(k):
                return wts_sb[:, k * 128:(k + 1) * 128]

            def prop(pi, ordering, consumer):
                """AG_A -> A gathers (all 20) -> AG_B -> per tile: B gather +
                S matmuls (A chunks then B chunks) -> consumer."""
                if pi > 0:
                    nc.gpsimd.collective_compute(
                        "AllGather", mybir.AluOpType.bypass,
                        replica_groups=[list(range(NCORES))],
                        ins=[zin_a.opt()], outs=[ztabA[pi].opt()],
                    )
                gbsA = []
                for g in range(NT):
                    gba = gpa.tile([128, GTA, F2], FP8, tag="gba")
                    nc.gpsimd.dma_gather(
                        gba[:], ztabA[pi][:],
                        idx_a[ordering][:, g * GCA:(g + 1) * GCA],
                        num_idxs=GTA * ET, num_idxs_reg=GTA * ET,
                        elem_size=F2, single_packet=False, queue_num=g % 4,
                    )
                    gbsA.append(gba)
                if pi > 0:
                    nc.gpsimd.collective_compute(
                        "AllGather", mybir.AluOpType.bypass,
                        replica_groups=[list(range(NCORES))],
                        ins=[zin_b.opt()], outs=[ztabB[pi].opt()],
                    )
                for g in range(NT):
                    gbb = gpb.tile([128, GTB, F2], FP8, tag="gbb")
                    nc.gpsimd.dma_gather(
                        gbb[:], ztabB[pi][:],
                        idx_b[ordering][:, g * GCB:(g + 1) * GCB],
                        num_idxs=GTB * ET, num_idxs_reg=GTB * ET,
                        elem_size=F2, single_packet=False, queue_num=g % 4,
                    )
                    ssb = spool.tile([128, GTS * ET], FP8, tag="schunk")
                    nc.sync.dma_start(ssb[:], s_dram[ordering][g])
                    ps = prop_ps.tile([128, F2], F32, tag="prop_ps")
                    for t in range(GTA):
                        nc.tensor.matmul(ps[:], ssb[:, t * ET:(t + 1) * ET],
                                         gbsA[g][:, t, :],
                                         start=(t == 0), stop=False)
                    for t in range(GTB):
                        nc.tensor.matmul(
                            ps[:], ssb[:, (GTA + t) * ET:(GTA + t + 1) * ET],
                            gbb[:, t, :],
                            start=False, stop=(t == GTB - 1))
                    consumer(g, ps)

            STT = nc.vector.scalar_tensor_tensor
            MUL = mybir.AluOpType.mult
            ADD = mybir.AluOpType.add
            COPY = mybir.ActivationFunctionType.Copy

            def smul(out_ap, in_ap, c_ap):
                """out = in * per-partition scalar, on the (idle) Scalar engine."""
                nc.scalar.activation(out_ap, in_ap, COPY, scale=c_ap)

            def zin_tile(g):
                """Push this dst tile's fresh z rows to its half-table input."""
                if g < NTA:
                    nc.sync.dma_start(zin_a[g * 128:(g + 1) * 128, :],
                                      z_own[:, g, :])
                else:
                    gg = g - NTA
                    nc.sync.dma_start(zin_b[gg * 128:(gg + 1) * 128, :],
                                      z_own[:, g, :])

            def b_consumer(ci):
                def consume(g, ps):
                    u_re, u_im = ps[:, 0:F], ps[:, F:F2]
                    # b = s.u + d.y
                    tmp3 = tmpp.tile([128, F], F32, tag="ctmp3")
                    smul(tmp3[:], u_re, col(ci, "s_re", g))
                    STT(tmp3[:], u_im, col(ci, "ns_im", g), tmp3[:], MUL, ADD)
                    STT(tmp3[:], y_re[:, g, :], col(ci, "d_re", g), tmp3[:], MUL, ADD)
                    STT(b_re[:, g, :], y_im[:, g, :], col(ci, "nd_im", g), tmp3[:], MUL, ADD)
                    tmp4 = tmpp.tile([128, F], F32, tag="ctmp4")
                    smul(tmp4[:], u_im, col(ci, "s_re", g))
                    STT(tmp4[:], u_re, col(ci, "s_im", g), tmp4[:], MUL, ADD)
                    STT(tmp4[:], y_im[:, g, :], col(ci, "d_re", g), tmp4[:], MUL, ADD)
                    STT(b_im[:, g, :], y_re[:, g, :], col(ci, "d_im", g), tmp4[:], MUL, ADD)
                    # first Jacobi gather operand: gs.u + gd.y == g.(s.u+d.y) = g.b
                    tmp = tmpp.tile([128, F], F32, tag="ctmp")
                    smul(tmp[:], b_re[:, g, :], col(ci, "g_re", g))
                    STT(z_own[:, g, 0:F], b_im[:, g, :], col(ci, "ng_im", g), tmp[:], MUL, ADD)
                    tmp2 = tmpp.tile([128, F], F32, tag="ctmp2")
                    smul(tmp2[:], b_im[:, g, :], col(ci, "g_re", g))
                    STT(z_own[:, g, F:F2], b_re[:, g, :], col(ci, "g_im", g), tmp2[:], MUL, ADD)
                    zin_tile(g)
                return consume

            def jacobi_consumer(ci, last, need_z):
                def consume(g, ps):
                    u_re, u_im = ps[:, 0:F], ps[:, F:F2]
                    # y = u + b   (this is yk)
                    nc.vector.tensor_tensor(y_re[:, g, :], u_re, b_re[:, g, :], ADD)
                    nc.vector.tensor_tensor(y_im[:, g, :], u_im, b_im[:, g, :], ADD)
                    if last:
                        if need_z:
                            # next B apply gathers z = y
                            smul(z_own[:, g, 0:F], y_re[:, g, :], 1.0)
                            smul(z_own[:, g, F:F2], y_im[:, g, :], 1.0)
                            zin_tile(g)
                        # bf16 staging for the Wc transposes
                        smul(yb[:, g, 0:F], y_re[:, g, :], 1.0)
                        smul(yb[:, g, F:F2], y_im[:, g, :], 1.0)
                    else:
                        # z = g (.) y   (next Jacobi gather operand)
                        tmp = tmpp.tile([128, F], F32, tag="ctmp")
                        smul(tmp[:], y_re[:, g, :], col(ci, "g_re", g))
                        STT(z_own[:, g, 0:F], y_im[:, g, :], col(ci, "ng_im", g), tmp[:], MUL, ADD)
                        tmp2 = tmpp.tile([128, F], F32, tag="ctmp2")
                        smul(tmp2[:], y_im[:, g, :], col(ci, "g_re", g))
                        STT(z_own[:, g, F:F2], y_re[:, g, :], col(ci, "g_im", g), tmp2[:], MUL, ADD)
                        zin_tile(g)
                return consume

            def transpose_to(dst, src_ap, t):
                """dst[:, t*128:(t+1)*128] = src_ap.T (both bf16)."""
                pt = tr_ps.tile([128, 128], BF16, tag="trps")
                nc.tensor.transpose(pt[:], src_ap, ident[:])
                nc.vector.tensor_copy(dst[:, t * 128:(t + 1) * 128], pt[:])

            def dense_chunks(lhs_ks, rhs_list, first):
                """out_acc[:, ch] (+)= sum_i lhsT(k_i) @ rhs_i[:, ch] (x2 if not first)."""
                nch = NT * F // 512
                for ch in range(nch):
                    sl = slice(ch * 512, (ch + 1) * 512)
                    ps = mm_ps.tile([128, 512], F32, tag="mmps")
                    for i, (k, rhs) in enumerate(zip(lhs_ks, rhs_list)):
                        nc.tensor.matmul(ps[:], wt(k), rhs[:, sl],
                                         start=(i == 0), stop=(i == len(lhs_ks) - 1))
                    if first:
                        nc.vector.tensor_copy(out_acc[:, sl], ps[:])
                    else:
                        STT(out_acc[:, sl], ps[:], 2.0, out_acc[:, sl], MUL, ADD)

            # ================= conv block =================
            pctr = [0]
            for ci in range(2):
                wbase = ci * 5
                dense_chunks([wbase + 0], [r_bf], first=True)

                for j in range(R):
                    prop(pctr[0], "O1", b_consumer(ci)); pctr[0] += 1
                    for it in range(K):
                        prop(pctr[0], "O2",
                             jacobi_consumer(ci, last=(it == K - 1),
                                             need_z=(j < R - 1)))
                        pctr[0] += 1
                    # yT from yb halves (bf16 copies of y)
                    for t in range(NT):
                        transpose_to(yT_re, yb[:, t, 0:F], t)
                        transpose_to(yT_im, yb[:, t, F:F2], t)
                    dense_chunks([wbase + 1 + 2 * j, wbase + 2 + 2 * j],
                                 [yT_re, yT_im], first=False)

                if ci == 0:
                    # relu -> bf16, transpose back to node-major, reseed state
                    nc.vector.tensor_scalar_max(r_bf[:], out_acc[:], 0.0)
                    nc.vector.memset(y_im[:], 0.0)
                    for t in range(NT):
                        pt = tr_ps.tile([128, 128], BF16, tag="trps")
                        nc.tensor.transpose(pt[:], r_bf[:, t * 128:(t + 1) * 128],
                                            ident[:])
                        nc.vector.tensor_copy(z_own[:, t, 0:F], pt[:])
                        nc.vector.memset(z_own[:, t, F:F2], 0.0)
                        nc.vector.tensor_copy(y_re[:, t, :], pt[:])
                        zin_tile(t)
                else:
                    # x2 = relu(out_acc) in place, feature-major f32 -> DRAM
                    nc.vector.tensor_scalar_max(out_acc[:], out_acc[:], 0.0)
                    nc.sync.dma_start(xfeat_out[:], out_acc[:])

    nc.compile()
    return nc


# ----------------------------------------------------------------------------
# entry point
# ----------------------------------------------------------------------------

def kernel(x, edge_index, W_real1, Wc1, W_real2, Wc2, h, alpha,
           pool_w, lin_W, lin_b):
    x = np.asarray(x, np.float32)
    edge_index = np.asarray(edge_index)
    row, col = edge_index[0].astype(np.int64), edge_index[1].astype(np.int64)

    GTA, GTB, tabs = _build_edge_tables(row, col)
    if "nc" not in _cache or _cache.get("GT") != (GTA, GTB):
        _cache["nc"] = _build(GTA, GTB)
        _cache["GT"] = (GTA, GTB)
    nc = _cache["nc"]

    deg = np.bincount(row, minlength=N).astype(np.float64)

    # per-node scale columns, both convs
    scal = np.zeros((NCORES, 128, 2 * NSCAL * NT), np.float32)
    for ci in range(2):
        sc = _scales(deg, float(np.asarray(h)[ci]), float(np.asarray(alpha)[ci]))
        for k, name in enumerate(SCAL_NAMES):
            cols = _shard_cols(sc[name].astype(np.float32))
            scal[:, :, (ci * NSCAL + k) * NT:(ci * NSCAL + k + 1) * NT] = cols

    # weights: lhsT layouts [cin, cout] bf16; imag pre-negated
    def T16(w):
        return np.ascontiguousarray(w.T).astype(ml_dtypes.bfloat16)
    wts = np.zeros((128, 10 * 128), ml_dtypes.bfloat16)
    packs = [T16(W_real1), T16(Wc1[0, :, :, 0]), T16(-Wc1[0, :, :, 1]),
             T16(Wc1[1, :, :, 0]), T16(-Wc1[1, :, :, 1]),
             T16(W_real2), T16(Wc2[0, :, :, 0]), T16(-Wc2[0, :, :, 1]),
             T16(Wc2[1, :, :, 0]), T16(-Wc2[1, :, :, 1])]
    for k, w in enumerate(packs):
        wts[:, k * 128:(k + 1) * 128] = w

    xn = _node_major(x)                                   # [NCORES,128,NT,F]
    xz = np.zeros((NCORES, 128, NT, F2), FP8NP)
    xz[:, :, :, :F] = xn.astype(FP8NP)
    y0 = xn.reshape(NCORES, 128, NT * F)
    # feature-major bf16 x per core: xt0[c][f, t*128+p] = x[node(t,p), f]
    xt0 = np.ascontiguousarray(
        xn.transpose(0, 3, 2, 1)                          # [c, F, NT, 128]
    ).reshape(NCORES, F, NT * 128).astype(ml_dtypes.bfloat16)
    # replicated full fp8 half-tables for prop 0 (skips the first AllGather):
    # half-row c*1280 + t*128 + p = node (c, t(+NTA), p), re||im
    full = xn.transpose(0, 2, 1, 3).reshape(NCORES, NT, 128, F)  # [c,t,p,F]
    zt0a = np.zeros((ZROWS_H, F2), FP8NP)
    zt0b = np.zeros((ZROWS_H, F2), FP8NP)
    zt0a[:, :F] = full[:, :NTA].reshape(ZROWS_H, F).astype(FP8NP)
    zt0b[:, :F] = full[:, NTA:].reshape(ZROWS_H, F).astype(FP8NP)

    ident = np.eye(128, dtype=ml_dtypes.bfloat16)

    (i1a, i1b, S1), (i2a, i2b, S2) = tabs["O1"], tabs["O2"]
    in_maps = []
    for c in range(NCORES):
        in_maps.append({
            "xz": xz[c], "xt0": xt0[c], "zt0a": zt0a, "zt0b": zt0b,
            "y0": y0[c],
            "i1a": i1a[c], "i1b": i1b[c], "i2a": i2a[c], "i2b": i2b[c],
            "s1": S1[c], "s2": S2[c],
            "scal": scal[c], "wts": wts, "ident": ident,
        })

    import os
    trace = os.environ.get("KERNEL_TRACE", "0") == "1"
    res = run_bass_kernel_spmd(nc, in_maps, core_ids=list(range(NCORES)),
                               trace=trace)
    _cache["last_results"] = res

    # unshard x2: xfeat[c][o, t*128+p] -> x2[c*2500 + t*128 + p, o]
    x2 = np.empty((N, HID), np.float32)
    for c in range(NCORES):
        xf = res.results[c]["xfeat"].reshape(128, NT * F)
        x2[c * NLOC:(c + 1) * NLOC] = xf.T[:NLOC]

    # host tail: tanh score, top-k (stable ties), weighted mean, linear
    pw = np.asarray(pool_w, np.float32)
    score = np.tanh((x2 @ pw) / np.linalg.norm(pw)).astype(np.float32)
    kpool = int(np.ceil(RATIO * N))
    idx = np.argsort(-score, kind="stable")[:kpool]
    x_sel = x2[idx] * score[idx][:, None]
    pooled = x_sel.mean(axis=0, keepdims=True).astype(np.float32)
    return (pooled @ np.asarray(lin_W, np.float32).T
            + np.asarray(lin_b, np.float32)).astype(np.float32)
